# revision 4
# baseline (speedup 1.0000x reference)
"""BitNet 4-layer MLP (8192x4096, ternary weights, int8-style activations)
on 8 Trainium2 NeuronCores — v2.

Strategy: pure data-parallel over the 8192-token dim (1024 tokens/core, no
collectives), activations transposed on chip ([feature, token]).  Weights are
ternary bf16 (exact), activations are int8-range integers in bf16 (exact),
PSUM accumulates fp32 (exact).  The 8192 N=512 bf16 matmuls per core are the
roofline (~213ns each); everything else is arranged to never stall the PE
FIFO:

  * LayerNorm stats are accumulated across output tiles on DVE/ACT
    (accS += h, accQ += Square(h)) so the PE only runs ONE f32r hi/lo
    ones-matmul pair per (layer, half) instead of 4 per output tile.
  * All auxiliary PE work (stats matmuls, mu/rstd broadcast matmuls) is
    emitted a few chunks INTO the next half's main-matmul stream, so its
    upstream DVE/ACT dependencies are always resolved before the PE FIFO
    reaches it.
  * The LN+quantize chain is fused to 4 DVE ops per tile (scale/offset rows
    folded with 1/in_scale, magic-number RNE quantization), spread 2 tiles
    per chunk across the next half's stream.
  * gamma==1/beta==0 (true for this model's inputs) removes the per-tile
    gamma/beta op; detected on host, with a generic fallback variant.
  * DRAM layouts are contiguous per DMA (half-major activations/outputs) to
    avoid the 1KB-row descriptor storm that made the baseline idle ~99us at
    startup.
  * Weights ship as fp8-e4m3 (ternary {-1,0,1} is exact in e4m3; the PE
    accepts fp8 stationary x bf16 moving and the result is bit-identical),
    halving weight DMA to 128MB/core.  Initial activations ship as int8 and
    are widened to bf16 on DVE, halving the startup-critical bytes.
  * ~160 tiny warmup matmuls run while the first DMAs land, so the HAM
    clock-gate reaches 2.4GHz before the real matmul stream starts.

Measured: 2139us (baseline) -> ~1817us on 8 axon-tunneled trn2 cores;
PE busy ~1790us vs a 1768us bf16-roofline for the 8192 N=512 matmuls.
"""

import numpy as np

NUM_CORES = 8
N_TOK, D = 8192, 4096
NUM_LAYERS = 4
P = 128                      # SBUF partitions
KT = D // P                  # 32 k-tiles per contraction
NLOC = N_TOK // NUM_CORES    # 1024 tokens per core
HALF = 512                   # token half-chunk (one PSUM bank @ fp32)
MAGIC = 12582912.0           # 1.5 * 2**23: fp32 add/sub does RNE-to-integer

_prog_cache = {}


def _install_drain_patch():
    """walrus CoreV3 rejects instructions carrying >~2 embedded sem waits
    ("Too many sync wait commands"). Tile's exit drain waits on the whole
    vector clock; spread its waits across trailing sync-engine nops."""
    import concourse.tile as tile
    import concourse.mybir as mybir
    from concourse.tile import ScopedClock

    if getattr(tile.TileContext, "_drain_patch_installed", False):
        return

    def _patched(self, tick_clock, wait_clock):
        nc = self.nc
        drain_inst = nc.sync.drain()
        wait_clock.add_sem_waits(
            drain_inst.ins, ScopedClock({None: tick_clock.global_clock})
        )
        si = drain_inst.ins.sync_info
        waits = list(si.on_wait or []) if si is not None else []
        if len(waits) > 1:
            si.on_wait = waits[:1]
            for w in waits[1:]:
                nop = nc.sync.nop(nofuse=True)
                nsi = nop.ins.sync_info
                if nsi is None:
                    nop.ins.sync_info = mybir.SyncInfo(on_wait=[w], on_update=[])
                else:
                    nsi.on_wait = [w]
        nc.all_engine_barrier()
        assert self.sems is not None
        popped = nc._tile_sem_poison_stack.pop()
        assert popped is self._sem_poison
        nc.clear_and_free_semaphores(list(self.sems.allocated().values()))
        nc.all_engine_barrier()

    tile.TileContext._drain_and_barrier = _patched
    tile.TileContext._drain_patch_installed = True


def _split_excess_waits(nc, maxw=1):
    """walrus's per-instruction sync-wait encodings hold few waits; hoist
    excess waits onto same-engine nops spliced immediately before the
    overloaded instruction (adjacent on the same queue, so ordering
    semantics are unchanged)."""
    import copy
    import concourse.mybir as mybir

    ctr = [0]
    proto = nc.sync.nop(nofuse=True)
    _NOP_PROTO = copy.deepcopy(proto.ins)
    _NOP_PROTO.sync_info = None

    def make_nop(proto_engine, waits):
        ctr[0] += 1
        nop = copy.deepcopy(_NOP_PROTO)
        nop.name = f"I-waitsplit-{ctr[0]}"
        nop.engine = proto_engine
        nop.sync_info = mybir.SyncInfo(on_wait=list(waits), on_update=[])
        return nop

    for bb in nc.m.functions[0].blocks:
        changed = False
        out = []
        for inst in bb.instructions:
            si = inst.sync_info
            waits = list(si.on_wait) if (si is not None and si.on_wait) else []
            if len(waits) > maxw and type(inst).__name__ != "InstISA":
                for i in range(0, len(waits) - maxw, maxw):
                    out.append(make_nop(inst.engine, waits[i:i + maxw]))
                si.on_wait = waits[len(waits) - maxw:]
                changed = True
            out.append(inst)
        if changed:
            bb.instructions = out
    return nc


def _build_program(s_deq, inv_in, fast_gb):
    """Build the per-core Bass program (identical across cores; data-parallel).

    s_deq[l]   = in_scale[l]*w_scale[l] as python floats (fp32-exact values)
    inv_in[l]  = 1/in_scale[l] likewise
    fast_gb    = True when gamma==1 and beta==0 (skip the per-tile op)
    """
    import concourse.bass as bass
    import concourse.mybir as mybir
    import concourse.tile as tile

    _install_drain_patch()
    dt = mybir.dt
    Alu = mybir.AluOpType
    Act = mybir.ActivationFunctionType

    nc = bass.Bass()
    W_d = nc.dram_tensor("wt", [NUM_LAYERS, KT, P, KT, P], dt.float8e4,
                         kind="ExternalInput")
    X_d = nc.dram_tensor("xq0", [2, KT, P, HALF], dt.int8,
                         kind="ExternalInput")
    O_d = nc.dram_tensor("out", [2, KT, P, HALF], dt.float32,
                         kind="ExternalOutput")
    if not fast_gb:
        G_d = nc.dram_tensor("gam", [NUM_LAYERS - 1, P, KT], dt.float32,
                             kind="ExternalInput")
        B_d = nc.dram_tensor("bet", [NUM_LAYERS - 1, P, KT], dt.float32,
                             kind="ExternalInput")

    f32, f32r, bf16 = dt.float32, dt.float32r, dt.bfloat16

    with tile.TileContext(nc) as tc:
        with (
            tc.tile_pool(name="xq", bufs=64) as xq_pool,
            tc.tile_pool(name="h", bufs=34) as h_pool,
            tc.tile_pool(name="w", bufs=4) as w_pool,
            tc.tile_pool(name="sq", bufs=3) as sq_pool,
            tc.tile_pool(name="acc", bufs=2) as acc_pool,
            tc.tile_pool(name="hr", bufs=4) as hr_pool,
            tc.tile_pool(name="bc", bufs=4) as bc_pool,
            tc.tile_pool(name="st", bufs=6) as st_pool,
            tc.tile_pool(name="gb", bufs=6) as gb_pool,
            tc.tile_pool(name="xi", bufs=8) as xi_pool,
            tc.tile_pool(name="const", bufs=1) as const_pool,
            tc.tile_pool(name="mmps", bufs=3, space="PSUM") as mm_ps,
            tc.tile_pool(name="stps", bufs=2, space="PSUM") as st_ps,
            tc.tile_pool(name="bcps", bufs=2, space="PSUM") as bc_ps,
        ):
            ones_f = const_pool.tile([P, 1], f32)
            nc.vector.memset(ones_f[:], 1.0)
            ones = const_pool.tile([P, 1], f32r)
            nc.vector.tensor_copy(ones[:], ones_f[:])
            eps = const_pool.tile([1, 1], f32)
            nc.vector.memset(eps[:], 1e-5)
            ones_row = const_pool.tile([1, P], f32)
            nc.vector.memset(ones_row[:], 1.0)
            ones_bf = const_pool.tile([P, 1], bf16)
            nc.vector.memset(ones_bf[:], 1.0)

            # state threaded between emission callbacks
            xq_tiles = {}     # (l, half, kt) -> bf16 [P, HALF]
            h_tiles = {}      # (l, half, ot) -> f32 [P, HALF]
            accs = {}         # (l, half) -> (accS, accQ)
            stps = {}         # (l, half) -> (S_ps, Q_ps)
            rows = {}         # (l, half) -> (a_row, b_row)
            bcs = {}          # (l, half) -> (aB, bB)
            gbt = {}          # l -> (gam [P,KT], bet [P,KT])

            # PE warmup: HAM un-throttles after ~3.4us of sustained matmul
            # activity; burn tiny matmuls on the const tile while the first
            # input DMAs are in flight so the real stream starts at 2.4GHz.
            warm_ps = st_ps.tile([1, 1], f32, tag="stps")
            for _ in range(160):
                nc.tensor.matmul(warm_ps[:], ones_bf[:], ones_bf[:],
                                 start=True, stop=True,
                                 skip_group_check=True)

            # first two weight tiles ahead of everything else: their
            # descriptors land at the head of every DMA queue, so the PE can
            # start chunk 0 as soon as the first xq tiles trickle in.
            pre_w = {}
            for ot in range(2):
                w = w_pool.tile([P, KT, P], dt.float8e4, tag="w")
                nc.sync.dma_start(w[:], W_d[0, ot])
                pre_w[ot] = w

            # initial activation DMAs: half 0 up-front (first chunks need all
            # 32 of them); half 1 interleaved into the first 32 chunks.
            def load_xq0(half, kt):
                ti = xi_pool.tile([P, HALF], dt.int8, tag="xi")
                nc.sync.dma_start(ti[:], X_d[half, kt])
                t = xq_pool.tile([P, HALF], bf16, tag="xq")
                nc.vector.tensor_copy(t[:], ti[:])
                xq_tiles[(0, half, kt)] = t

            for kt in range(KT):
                load_xq0(0, kt)

            if not fast_gb:
                for l in range(NUM_LAYERS - 1):
                    g = gb_pool.tile([P, KT], f32, tag="gb")
                    nc.sync.dma_start(g[:], G_d[l])
                    b = gb_pool.tile([P, KT], f32, tag="gb")
                    nc.sync.dma_start(b[:], B_d[l])
                    gbt[l] = (g, b)

            def emit_chunk(l, half, ot):
                if l == 0 and half == 0:
                    # stream in half 1 of the initial activations
                    load_xq0(1, ot)
                if l == 0 and half == 0 and ot in pre_w:
                    w = pre_w.pop(ot)
                else:
                    w = w_pool.tile([P, KT, P], dt.float8e4, tag="w")
                    nc.sync.dma_start(w[:], W_d[l, ot])
                ps = mm_ps.tile([P, HALF], f32, tag="mmps")
                for kt in range(KT):
                    nc.tensor.matmul(
                        ps[:], w[:, kt, :], xq_tiles[(l, half, kt)][:],
                        start=(kt == 0), stop=(kt == KT - 1),
                        skip_group_check=True)
                if l < NUM_LAYERS - 1:
                    h_t = h_pool.tile([P, HALF], f32, tag="h")
                    nc.scalar.activation(h_t[:], ps[:], Act.Relu,
                                         scale=float(s_deq[l]))
                    sq = sq_pool.tile([P, HALF], f32, tag="sq")
                    nc.scalar.activation(sq[:], h_t[:], Act.Square)
                    if ot == 0:
                        accS = acc_pool.tile([P, HALF], f32, tag="accS")
                        nc.vector.tensor_copy(accS[:], h_t[:])
                        accQ = acc_pool.tile([P, HALF], f32, tag="accQ")
                        nc.vector.tensor_copy(accQ[:], sq[:])
                        accs[(l, half)] = (accS, accQ)
                    else:
                        accS, accQ = accs[(l, half)]
                        nc.vector.tensor_tensor(accS[:], accS[:], h_t[:],
                                                op=Alu.add)
                        nc.vector.tensor_tensor(accQ[:], accQ[:], sq[:],
                                                op=Alu.add)
                    h_tiles[(l, half, ot)] = h_t
                else:
                    h_t = h_pool.tile([P, HALF], f32, tag="h")
                    nc.scalar.activation(h_t[:], ps[:], Act.Copy,
                                         scale=float(s_deq[l]))
                    nc.sync.dma_start(O_d[half, ot], h_t[:])

            def emit_stats(l, half):
                # partition-dim sums of accS/accQ via f32r hi/lo ones-matmuls
                # (full fp32 accuracy, 4 PE instructions total)
                accS, accQ = accs.pop((l, half))
                S_ps = st_ps.tile([1, HALF], f32, tag="stps")
                Q_ps = st_ps.tile([1, HALF], f32, tag="stps")
                for acc, ps_ in ((accS, S_ps), (accQ, Q_ps)):
                    hi = hr_pool.tile([P, HALF], f32r, tag="hr")
                    nc.vector.tensor_copy(hi[:], acc[:])
                    lo = hr_pool.tile([P, HALF], f32r, tag="hr")
                    nc.vector.tensor_tensor(lo[:], acc[:],
                                            hi[:].bitcast(f32),
                                            op=Alu.subtract)
                    nc.tensor.matmul(ps_[:], ones[:], hi[:],
                                     start=True, stop=False,
                                     skip_group_check=True)
                    nc.tensor.matmul(ps_[:], ones[:], lo[:],
                                     start=False, stop=True,
                                     skip_group_check=True)
                stps[(l, half)] = (S_ps, Q_ps)

            def emit_rows(l, half):
                S_ps, Q_ps = stps.pop((l, half))
                mu = st_pool.tile([1, HALF], f32, tag="st")
                nc.vector.tensor_scalar_mul(mu[:], S_ps[:], 1.0 / D)
                q = st_pool.tile([1, HALF], f32, tag="st")
                nc.vector.tensor_scalar_mul(q[:], Q_ps[:], 1.0 / D)
                var = st_pool.tile([1, HALF], f32, tag="st")
                nc.vector.tensor_tensor(var[:], mu[:], mu[:], op=Alu.mult)
                nc.vector.tensor_tensor(var[:], q[:], var[:], op=Alu.subtract)
                std = st_pool.tile([1, HALF], f32, tag="st")
                nc.scalar.activation(std[:], var[:], Act.Sqrt, bias=eps[:])
                rstd = st_pool.tile([1, HALF], f32, tag="st")
                nc.vector.reciprocal(rstd[:], std[:])
                a = st_pool.tile([1, HALF], f32, tag="st")
                if fast_gb:
                    # fold 1/in_scale of the next layer into the LN scale
                    nc.vector.tensor_scalar_mul(a[:], rstd[:],
                                                float(inv_in[l + 1]))
                else:
                    nc.vector.tensor_copy(a[:], rstd[:])
                b = st_pool.tile([1, HALF], f32, tag="st")
                nc.vector.tensor_tensor(b[:], mu[:], a[:], op=Alu.mult)
                rows[(l, half)] = (a, b)

            def emit_bcast(l, half):
                a, b = rows.pop((l, half))
                out = []
                for r in (a, b):
                    r_ps = bc_ps.tile([P, HALF], f32, tag="bcps")
                    nc.tensor.matmul(r_ps[:], ones_row[:], r[:],
                                     start=True, stop=True,
                                     skip_group_check=True)
                    rB = bc_pool.tile([P, HALF], f32, tag="bc")
                    nc.scalar.activation(rB[:], r_ps[:], Act.Copy)
                    out.append(rB)
                bcs[(l, half)] = tuple(out)

            def emit_ln(l, half, fts):
                aB, bB = bcs[(l, half)]
                inv = float(inv_in[l + 1])
                for ft in fts:
                    h_t = h_tiles.pop((l, half, ft))
                    nc.vector.tensor_tensor(h_t[:], h_t[:], aB[:],
                                            op=Alu.mult)
                    nc.vector.tensor_tensor(h_t[:], h_t[:], bB[:],
                                            op=Alu.subtract)
                    if not fast_gb:
                        g, bta = gbt[l]
                        nc.vector.tensor_scalar(h_t[:], h_t[:],
                                                g[:, ft:ft + 1],
                                                bta[:, ft:ft + 1],
                                                op0=Alu.mult, op1=Alu.add)
                        nc.vector.tensor_scalar(h_t[:], h_t[:], inv, MAGIC,
                                                op0=Alu.mult, op1=Alu.add)
                        nc.vector.tensor_scalar(h_t[:], h_t[:], MAGIC + 127.0,
                                                MAGIC - 128.0, op0=Alu.min,
                                                op1=Alu.max)
                        xq_t = xq_pool.tile([P, HALF], bf16, tag="xq")
                        nc.vector.tensor_scalar_add(xq_t[:], h_t[:], -MAGIC)
                    else:
                        nc.vector.tensor_scalar(h_t[:], h_t[:], MAGIC,
                                                MAGIC + 127.0,
                                                op0=Alu.add, op1=Alu.min)
                        xq_t = xq_pool.tile([P, HALF], bf16, tag="xq")
                        nc.vector.tensor_scalar(xq_t[:], h_t[:],
                                                MAGIC - 128.0, MAGIC,
                                                op0=Alu.max, op1=Alu.subtract)
                    xq_tiles[(l + 1, half, ft)] = xq_t

            # emission schedule: 256 main chunks with aux work interleaved a
            # few chunks into the following stream so the PE FIFO never waits
            chunks = [(l, half, ot)
                      for l in range(NUM_LAYERS)
                      for half in range(2)
                      for ot in range(KT)]
            aux = {}
            for l in range(NUM_LAYERS - 1):
                for half in range(2):
                    g_end = (l * 2 + half) * KT + KT - 1
                    aux.setdefault(g_end + 2, []).append(
                        lambda l=l, h=half: emit_stats(l, h))
                    aux.setdefault(g_end + 3, []).append(
                        lambda l=l, h=half: emit_rows(l, h))
                    aux.setdefault(g_end + 4, []).append(
                        lambda l=l, h=half: emit_bcast(l, h))
                    for i in range(16):
                        aux.setdefault(g_end + 5 + i, []).append(
                            lambda l=l, h=half, i=i:
                                emit_ln(l, h, [2 * i, 2 * i + 1]))

            # aux goes BEFORE its chunk: its DVE ops have ready inputs, while
            # the chunk's acc-adds are gated on the chunk's matmul group —
            # emitting them last keeps the (strict-FIFO) DVE from idling on
            # them with runnable LN work queued behind.
            for g, (l, half, ot) in enumerate(chunks):
                for fn in aux.pop(g, []):
                    fn()
                emit_chunk(l, half, ot)
            assert not aux, f"unemitted aux work: {sorted(aux)}"

    _split_excess_waits(nc)
    return nc


def kernel(x, Ws, w_scales, in_scales, gammas, betas, _trace=False):
    import ml_dtypes
    from concourse.bass_utils import run_bass_kernel_spmd

    f32 = np.float32
    C = f32(MAGIC)
    x = np.asarray(x, f32)
    Ws = np.asarray(Ws, f32)
    w_scales = np.asarray(w_scales, f32)
    in_scales = np.asarray(in_scales, f32)
    gammas = np.asarray(gammas, f32)
    betas = np.asarray(betas, f32)

    # ---- host prep (offline-weight-style preprocessing) ----
    # ternary quantize weights; XLA divides by reciprocal-multiply and
    # rounds nearest-even, both reproduced here bitwise.
    WT = np.empty((NUM_LAYERS, KT, P, KT, P), ml_dtypes.float8_e4m3fn)
    for l in range(NUM_LAYERS):
        wq = ((Ws[l] * (f32(1.0) / w_scales[l])) + C) - C
        wq = np.clip(wq, -1.0, 1.0).astype(f32)
        # WT[l, ot, kp, kt, o] = wq[ot*128+o, kt*128+kp]
        t = wq.reshape(KT, P, KT, P)          # [ot, o, kt, kp]
        WT[l] = t.transpose(0, 3, 2, 1).astype(ml_dtypes.float8_e4m3fn)

    xq0 = ((x * (f32(1.0) / in_scales[0])) + C) - C
    xq0 = np.clip(xq0, -128.0, 127.0).astype(f32)
    xT = np.ascontiguousarray(xq0.T)           # [D, N]

    fast_gb = bool(np.all(gammas == 1.0) and np.all(betas == 0.0))
    s_deq = [float(in_scales[l] * w_scales[l]) for l in range(NUM_LAYERS)]
    inv_in = [float(f32(1.0) / in_scales[l]) for l in range(NUM_LAYERS)]

    key = (tuple(s_deq), tuple(inv_in), fast_gb)
    if key not in _prog_cache:
        _prog_cache[key] = _build_program(s_deq, inv_in, fast_gb)
    nc = _prog_cache[key]

    in_maps = []
    for c in range(NUM_CORES):
        xs = xT[:, c * NLOC:(c + 1) * NLOC]        # [D, NLOC]
        xh = np.stack([
            xs[:, h * HALF:(h + 1) * HALF].reshape(KT, P, HALF)
            for h in range(2)])                     # [2, KT, P, HALF]
        im = {"wt": WT,
              "xq0": np.ascontiguousarray(xh).astype(np.int8)}
        if not fast_gb:
            im["gam"] = np.ascontiguousarray(
                gammas.reshape(NUM_LAYERS - 1, KT, P).transpose(0, 2, 1))
            im["bet"] = np.ascontiguousarray(
                betas.reshape(NUM_LAYERS - 1, KT, P).transpose(0, 2, 1))
        in_maps.append(im)

    res = run_bass_kernel_spmd(nc, in_maps, list(range(NUM_CORES)),
                               trace=_trace)
    if _trace:
        kernel.last_exec_ns = res.exec_time_ns

    # gather: out[c] is [2, KT, P, HALF] -> per-core [D, NLOC] -> [N, D]
    full = np.empty((N_TOK, D), np.float32)
    for c in range(NUM_CORES):
        o = np.asarray(res.results[c]["out"], np.float32)  # [2, KT, P, HALF]
        for h in range(2):
            rows = o[h].reshape(D, HALF).T                 # [HALF, D]
            full[c * NLOC + h * HALF:c * NLOC + (h + 1) * HALF] = rows
    return full


kernel.last_exec_ns = None


# revision 5
# speedup vs baseline: 1.0021x; 1.0021x over previous
"""BitNet 4-layer MLP (8192x4096, ternary weights, int8-style activations)
on 8 Trainium2 NeuronCores — v2.

Strategy: pure data-parallel over the 8192-token dim (1024 tokens/core, no
collectives), activations transposed on chip ([feature, token]).  Weights are
ternary bf16 (exact), activations are int8-range integers in bf16 (exact),
PSUM accumulates fp32 (exact).  The 8192 N=512 bf16 matmuls per core are the
roofline (~213ns each); everything else is arranged to never stall the PE
FIFO:

  * LayerNorm stats are accumulated across output tiles on DVE/ACT
    (accS += h, accQ += Square(h)) so the PE only runs ONE f32r hi/lo
    ones-matmul pair per (layer, half) instead of 4 per output tile.
  * All auxiliary PE work (stats matmuls, mu/rstd broadcast matmuls) is
    emitted a few chunks INTO the next half's main-matmul stream, so its
    upstream DVE/ACT dependencies are always resolved before the PE FIFO
    reaches it.
  * The LN+quantize chain is fused to 4 DVE ops per tile (scale/offset rows
    folded with 1/in_scale, magic-number RNE quantization), spread 2 tiles
    per chunk across the next half's stream.
  * gamma==1/beta==0 (true for this model's inputs) removes the per-tile
    gamma/beta op; detected on host, with a generic fallback variant.
  * DRAM layouts are contiguous per DMA (half-major activations/outputs) to
    avoid the 1KB-row descriptor storm that made the baseline idle ~99us at
    startup.
"""

import numpy as np

NUM_CORES = 8
N_TOK, D = 8192, 4096
NUM_LAYERS = 4
P = 128                      # SBUF partitions
KT = D // P                  # 32 k-tiles per contraction
NLOC = N_TOK // NUM_CORES    # 1024 tokens per core
HALF = 512                   # token half-chunk (one PSUM bank @ fp32)
MAGIC = 12582912.0           # 1.5 * 2**23: fp32 add/sub does RNE-to-integer

_prog_cache = {}


def _install_drain_patch():
    """walrus CoreV3 rejects instructions carrying >~2 embedded sem waits
    ("Too many sync wait commands"). Tile's exit drain waits on the whole
    vector clock; spread its waits across trailing sync-engine nops."""
    import concourse.tile as tile
    import concourse.mybir as mybir
    from concourse.tile import ScopedClock

    if getattr(tile.TileContext, "_drain_patch_installed", False):
        return

    def _patched(self, tick_clock, wait_clock):
        nc = self.nc
        drain_inst = nc.sync.drain()
        wait_clock.add_sem_waits(
            drain_inst.ins, ScopedClock({None: tick_clock.global_clock})
        )
        si = drain_inst.ins.sync_info
        waits = list(si.on_wait or []) if si is not None else []
        if len(waits) > 1:
            si.on_wait = waits[:1]
            for w in waits[1:]:
                nop = nc.sync.nop(nofuse=True)
                nsi = nop.ins.sync_info
                if nsi is None:
                    nop.ins.sync_info = mybir.SyncInfo(on_wait=[w], on_update=[])
                else:
                    nsi.on_wait = [w]
        nc.all_engine_barrier()
        assert self.sems is not None
        popped = nc._tile_sem_poison_stack.pop()
        assert popped is self._sem_poison
        nc.clear_and_free_semaphores(list(self.sems.allocated().values()))
        nc.all_engine_barrier()

    tile.TileContext._drain_and_barrier = _patched
    tile.TileContext._drain_patch_installed = True


def _split_excess_waits(nc, maxw=1):
    """walrus's per-instruction sync-wait encodings hold few waits; hoist
    excess waits onto same-engine nops spliced immediately before the
    overloaded instruction (adjacent on the same queue, so ordering
    semantics are unchanged)."""
    import copy
    import concourse.mybir as mybir

    ctr = [0]
    proto = nc.sync.nop(nofuse=True)
    _NOP_PROTO = copy.deepcopy(proto.ins)
    _NOP_PROTO.sync_info = None

    def make_nop(proto_engine, waits):
        ctr[0] += 1
        nop = copy.deepcopy(_NOP_PROTO)
        nop.name = f"I-waitsplit-{ctr[0]}"
        nop.engine = proto_engine
        nop.sync_info = mybir.SyncInfo(on_wait=list(waits), on_update=[])
        return nop

    for bb in nc.m.functions[0].blocks:
        changed = False
        out = []
        for inst in bb.instructions:
            si = inst.sync_info
            waits = list(si.on_wait) if (si is not None and si.on_wait) else []
            if len(waits) > maxw and type(inst).__name__ != "InstISA":
                for i in range(0, len(waits) - maxw, maxw):
                    out.append(make_nop(inst.engine, waits[i:i + maxw]))
                si.on_wait = waits[len(waits) - maxw:]
                changed = True
            out.append(inst)
        if changed:
            bb.instructions = out
    return nc


def _build_program(s_deq, inv_in, fast_gb):
    """Build the per-core Bass program (identical across cores; data-parallel).

    s_deq[l]   = in_scale[l]*w_scale[l] as python floats (fp32-exact values)
    inv_in[l]  = 1/in_scale[l] likewise
    fast_gb    = True when gamma==1 and beta==0 (skip the per-tile op)
    """
    import concourse.bass as bass
    import concourse.mybir as mybir
    import concourse.tile as tile

    _install_drain_patch()
    dt = mybir.dt
    Alu = mybir.AluOpType
    Act = mybir.ActivationFunctionType

    nc = bass.Bass()
    W_d = nc.dram_tensor("wt", [NUM_LAYERS, KT, P, KT, P], dt.float8e4,
                         kind="ExternalInput")
    X_d = nc.dram_tensor("xq0", [2, KT, P, HALF], dt.int8,
                         kind="ExternalInput")
    O_d = nc.dram_tensor("out", [2, KT, P, HALF], dt.float32,
                         kind="ExternalOutput")
    if not fast_gb:
        G_d = nc.dram_tensor("gam", [NUM_LAYERS - 1, P, KT], dt.float32,
                             kind="ExternalInput")
        B_d = nc.dram_tensor("bet", [NUM_LAYERS - 1, P, KT], dt.float32,
                             kind="ExternalInput")

    f32, f32r, bf16 = dt.float32, dt.float32r, dt.bfloat16

    with tile.TileContext(nc) as tc:
        with (
            tc.tile_pool(name="xq", bufs=64) as xq_pool,
            tc.tile_pool(name="h", bufs=34) as h_pool,
            tc.tile_pool(name="w", bufs=8) as w_pool,
            tc.tile_pool(name="sq", bufs=3) as sq_pool,
            tc.tile_pool(name="acc", bufs=2) as acc_pool,
            tc.tile_pool(name="hr", bufs=4) as hr_pool,
            tc.tile_pool(name="bc", bufs=4) as bc_pool,
            tc.tile_pool(name="st", bufs=6) as st_pool,
            tc.tile_pool(name="gb", bufs=6) as gb_pool,
            tc.tile_pool(name="xi", bufs=8) as xi_pool,
            tc.tile_pool(name="const", bufs=1) as const_pool,
            tc.tile_pool(name="mmps", bufs=3, space="PSUM") as mm_ps,
            tc.tile_pool(name="stps", bufs=2, space="PSUM") as st_ps,
            tc.tile_pool(name="bcps", bufs=2, space="PSUM") as bc_ps,
        ):
            ones_f = const_pool.tile([P, 1], f32)
            nc.vector.memset(ones_f[:], 1.0)
            ones = const_pool.tile([P, 1], f32r)
            nc.vector.tensor_copy(ones[:], ones_f[:])
            eps = const_pool.tile([1, 1], f32)
            nc.vector.memset(eps[:], 1e-5)
            ones_row = const_pool.tile([1, P], f32)
            nc.vector.memset(ones_row[:], 1.0)
            ones_bf = const_pool.tile([P, 1], bf16)
            nc.vector.memset(ones_bf[:], 1.0)

            # state threaded between emission callbacks
            xq_tiles = {}     # (l, half, kt) -> bf16 [P, HALF]
            h_tiles = {}      # (l, half, ot) -> f32 [P, HALF]
            accs = {}         # (l, half) -> (accS, accQ)
            stps = {}         # (l, half) -> (S_ps, Q_ps)
            rows = {}         # (l, half) -> (a_row, b_row)
            bcs = {}          # (l, half) -> (aB, bB)
            gbt = {}          # l -> (gam [P,KT], bet [P,KT])

            # PE warmup: HAM un-throttles after ~3.4us of sustained matmul
            # activity; burn tiny matmuls on the const tile while the first
            # input DMAs are in flight so the real stream starts at 2.4GHz.
            warm_ps = st_ps.tile([1, 1], f32, tag="stps")
            for _ in range(220):
                nc.tensor.matmul(warm_ps[:], ones_bf[:], ones_bf[:],
                                 start=True, stop=True,
                                 skip_group_check=True)

            # first two weight tiles ahead of everything else: their
            # descriptors land at the head of every DMA queue, so the PE can
            # start chunk 0 as soon as the first xq tiles trickle in.
            KH = KT // 2
            def load_w(l, ot):
                subs = []
                for j in range(2):
                    ws = w_pool.tile([P, KH, P], dt.float8e4, tag="w")
                    nc.sync.dma_start(ws[:], W_d[l, ot, :, j * KH:(j + 1) * KH])
                    subs.append(ws)
                return subs

            pre_w = {}

            # initial activation DMAs: half 0 up-front (first chunks need all
            # 32 of them); half 1 interleaved into the first 32 chunks.
            def load_xq0(half, kt):
                ti = xi_pool.tile([P, HALF], dt.int8, tag="xi")
                nc.sync.dma_start(ti[:], X_d[half, kt])
                t = xq_pool.tile([P, HALF], bf16, tag="xq")
                nc.vector.tensor_copy(t[:], ti[:])
                xq_tiles[(0, half, kt)] = t

            # interleave the startup DMAs along the consumption order:
            # first weight half, a few xq tiles, second half, the rest.
            w00a = w_pool.tile([P, KH, P], dt.float8e4, tag="w")
            nc.sync.dma_start(w00a[:], W_d[0, 0, :, 0:KH])
            for kt in range(4):
                load_xq0(0, kt)
            w00b = w_pool.tile([P, KH, P], dt.float8e4, tag="w")
            nc.sync.dma_start(w00b[:], W_d[0, 0, :, KH:KT])
            for kt in range(4, KT):
                load_xq0(0, kt)
            pre_w[0] = [w00a, w00b]
            pre_w[1] = load_w(0, 1)

            if not fast_gb:
                for l in range(NUM_LAYERS - 1):
                    g = gb_pool.tile([P, KT], f32, tag="gb")
                    nc.sync.dma_start(g[:], G_d[l])
                    b = gb_pool.tile([P, KT], f32, tag="gb")
                    nc.sync.dma_start(b[:], B_d[l])
                    gbt[l] = (g, b)

            def emit_chunk(l, half, ot):
                if l == 0 and half == 0:
                    # stream in half 1 of the initial activations
                    load_xq0(1, ot)
                if l == 0 and half == 0 and ot in pre_w:
                    w = pre_w.pop(ot)
                else:
                    w = load_w(l, ot)
                ps = mm_ps.tile([P, HALF], f32, tag="mmps")
                for kt in range(KT):
                    nc.tensor.matmul(
                        ps[:], w[kt // KH][:, kt % KH, :],
                        xq_tiles[(l, half, kt)][:],
                        start=(kt == 0), stop=(kt == KT - 1),
                        skip_group_check=True)
                if l < NUM_LAYERS - 1:
                    h_t = h_pool.tile([P, HALF], f32, tag="h")
                    nc.scalar.activation(h_t[:], ps[:], Act.Relu,
                                         scale=float(s_deq[l]))
                    sq = sq_pool.tile([P, HALF], f32, tag="sq")
                    nc.scalar.activation(sq[:], h_t[:], Act.Square)
                    if ot == 0:
                        accS = acc_pool.tile([P, HALF], f32, tag="accS")
                        nc.vector.tensor_copy(accS[:], h_t[:])
                        accQ = acc_pool.tile([P, HALF], f32, tag="accQ")
                        nc.vector.tensor_copy(accQ[:], sq[:])
                        accs[(l, half)] = (accS, accQ)
                    else:
                        accS, accQ = accs[(l, half)]
                        nc.vector.tensor_tensor(accS[:], accS[:], h_t[:],
                                                op=Alu.add)
                        nc.vector.tensor_tensor(accQ[:], accQ[:], sq[:],
                                                op=Alu.add)
                    h_tiles[(l, half, ot)] = h_t
                else:
                    h_t = h_pool.tile([P, HALF], f32, tag="h")
                    nc.scalar.activation(h_t[:], ps[:], Act.Copy,
                                         scale=float(s_deq[l]))
                    nc.sync.dma_start(O_d[half, ot], h_t[:])

            def emit_stats(l, half):
                # partition-dim sums of accS/accQ via f32r hi/lo ones-matmuls
                # (full fp32 accuracy, 4 PE instructions total)
                accS, accQ = accs.pop((l, half))
                S_ps = st_ps.tile([1, HALF], f32, tag="stps")
                Q_ps = st_ps.tile([1, HALF], f32, tag="stps")
                for acc, ps_ in ((accS, S_ps), (accQ, Q_ps)):
                    hi = hr_pool.tile([P, HALF], f32r, tag="hr")
                    nc.vector.tensor_copy(hi[:], acc[:])
                    lo = hr_pool.tile([P, HALF], f32r, tag="hr")
                    nc.vector.tensor_tensor(lo[:], acc[:],
                                            hi[:].bitcast(f32),
                                            op=Alu.subtract)
                    nc.tensor.matmul(ps_[:], ones[:], hi[:],
                                     start=True, stop=False,
                                     skip_group_check=True)
                    nc.tensor.matmul(ps_[:], ones[:], lo[:],
                                     start=False, stop=True,
                                     skip_group_check=True)
                stps[(l, half)] = (S_ps, Q_ps)

            def emit_rows(l, half):
                S_ps, Q_ps = stps.pop((l, half))
                mu = st_pool.tile([1, HALF], f32, tag="st")
                nc.vector.tensor_scalar_mul(mu[:], S_ps[:], 1.0 / D)
                q = st_pool.tile([1, HALF], f32, tag="st")
                nc.vector.tensor_scalar_mul(q[:], Q_ps[:], 1.0 / D)
                var = st_pool.tile([1, HALF], f32, tag="st")
                nc.vector.tensor_tensor(var[:], mu[:], mu[:], op=Alu.mult)
                nc.vector.tensor_tensor(var[:], q[:], var[:], op=Alu.subtract)
                std = st_pool.tile([1, HALF], f32, tag="st")
                nc.scalar.activation(std[:], var[:], Act.Sqrt, bias=eps[:])
                rstd = st_pool.tile([1, HALF], f32, tag="st")
                nc.vector.reciprocal(rstd[:], std[:])
                a = st_pool.tile([1, HALF], f32, tag="st")
                if fast_gb:
                    # fold 1/in_scale of the next layer into the LN scale
                    nc.vector.tensor_scalar_mul(a[:], rstd[:],
                                                float(inv_in[l + 1]))
                else:
                    nc.vector.tensor_copy(a[:], rstd[:])
                b = st_pool.tile([1, HALF], f32, tag="st")
                nc.vector.tensor_tensor(b[:], mu[:], a[:], op=Alu.mult)
                rows[(l, half)] = (a, b)

            def emit_bcast(l, half):
                a, b = rows.pop((l, half))
                out = []
                for r in (a, b):
                    r_ps = bc_ps.tile([P, HALF], f32, tag="bcps")
                    nc.tensor.matmul(r_ps[:], ones_row[:], r[:],
                                     start=True, stop=True,
                                     skip_group_check=True)
                    rB = bc_pool.tile([P, HALF], f32, tag="bc")
                    nc.scalar.activation(rB[:], r_ps[:], Act.Copy)
                    out.append(rB)
                bcs[(l, half)] = tuple(out)

            def emit_ln(l, half, fts):
                aB, bB = bcs[(l, half)]
                inv = float(inv_in[l + 1])
                for ft in fts:
                    h_t = h_tiles.pop((l, half, ft))
                    nc.vector.tensor_tensor(h_t[:], h_t[:], aB[:],
                                            op=Alu.mult)
                    nc.vector.tensor_tensor(h_t[:], h_t[:], bB[:],
                                            op=Alu.subtract)
                    if not fast_gb:
                        g, bta = gbt[l]
                        nc.vector.tensor_scalar(h_t[:], h_t[:],
                                                g[:, ft:ft + 1],
                                                bta[:, ft:ft + 1],
                                                op0=Alu.mult, op1=Alu.add)
                        nc.vector.tensor_scalar(h_t[:], h_t[:], inv, MAGIC,
                                                op0=Alu.mult, op1=Alu.add)
                        nc.vector.tensor_scalar(h_t[:], h_t[:], MAGIC + 127.0,
                                                MAGIC - 128.0, op0=Alu.min,
                                                op1=Alu.max)
                        xq_t = xq_pool.tile([P, HALF], bf16, tag="xq")
                        nc.vector.tensor_scalar_add(xq_t[:], h_t[:], -MAGIC)
                    else:
                        nc.vector.tensor_scalar(h_t[:], h_t[:], MAGIC,
                                                MAGIC + 127.0,
                                                op0=Alu.add, op1=Alu.min)
                        xq_t = xq_pool.tile([P, HALF], bf16, tag="xq")
                        nc.vector.tensor_scalar(xq_t[:], h_t[:],
                                                MAGIC - 128.0, MAGIC,
                                                op0=Alu.max, op1=Alu.subtract)
                    xq_tiles[(l + 1, half, ft)] = xq_t

            # emission schedule: 256 main chunks with aux work interleaved a
            # few chunks into the following stream so the PE FIFO never waits
            chunks = [(l, half, ot)
                      for l in range(NUM_LAYERS)
                      for half in range(2)
                      for ot in range(KT)]
            aux = {}
            for l in range(NUM_LAYERS - 1):
                for half in range(2):
                    g_end = (l * 2 + half) * KT + KT - 1
                    aux.setdefault(g_end + 2, []).append(
                        lambda l=l, h=half: emit_stats(l, h))
                    aux.setdefault(g_end + 3, []).append(
                        lambda l=l, h=half: emit_rows(l, h))
                    aux.setdefault(g_end + 4, []).append(
                        lambda l=l, h=half: emit_bcast(l, h))
                    for i in range(16):
                        aux.setdefault(g_end + 5 + i, []).append(
                            lambda l=l, h=half, i=i:
                                emit_ln(l, h, [2 * i, 2 * i + 1]))

            # aux goes BEFORE its chunk: its DVE ops have ready inputs, while
            # the chunk's acc-adds are gated on the chunk's matmul group —
            # emitting them last keeps the (strict-FIFO) DVE from idling on
            # them with runnable LN work queued behind.
            for g, (l, half, ot) in enumerate(chunks):
                for fn in aux.pop(g, []):
                    fn()
                emit_chunk(l, half, ot)
            assert not aux, f"unemitted aux work: {sorted(aux)}"

    _split_excess_waits(nc)
    return nc


def kernel(x, Ws, w_scales, in_scales, gammas, betas, _trace=False):
    import ml_dtypes
    from concourse.bass_utils import run_bass_kernel_spmd

    f32 = np.float32
    C = f32(MAGIC)
    x = np.asarray(x, f32)
    Ws = np.asarray(Ws, f32)
    w_scales = np.asarray(w_scales, f32)
    in_scales = np.asarray(in_scales, f32)
    gammas = np.asarray(gammas, f32)
    betas = np.asarray(betas, f32)

    # ---- host prep (offline-weight-style preprocessing) ----
    # ternary quantize weights; XLA divides by reciprocal-multiply and
    # rounds nearest-even, both reproduced here bitwise.
    WT = np.empty((NUM_LAYERS, KT, P, KT, P), ml_dtypes.float8_e4m3fn)
    for l in range(NUM_LAYERS):
        wq = ((Ws[l] * (f32(1.0) / w_scales[l])) + C) - C
        wq = np.clip(wq, -1.0, 1.0).astype(f32)
        # WT[l, ot, kp, kt, o] = wq[ot*128+o, kt*128+kp]
        t = wq.reshape(KT, P, KT, P)          # [ot, o, kt, kp]
        WT[l] = t.transpose(0, 3, 2, 1).astype(ml_dtypes.float8_e4m3fn)

    xq0 = ((x * (f32(1.0) / in_scales[0])) + C) - C
    xq0 = np.clip(xq0, -128.0, 127.0).astype(f32)
    xT = np.ascontiguousarray(xq0.T)           # [D, N]

    fast_gb = bool(np.all(gammas == 1.0) and np.all(betas == 0.0))
    s_deq = [float(in_scales[l] * w_scales[l]) for l in range(NUM_LAYERS)]
    inv_in = [float(f32(1.0) / in_scales[l]) for l in range(NUM_LAYERS)]

    key = (tuple(s_deq), tuple(inv_in), fast_gb)
    if key not in _prog_cache:
        _prog_cache[key] = _build_program(s_deq, inv_in, fast_gb)
    nc = _prog_cache[key]

    in_maps = []
    for c in range(NUM_CORES):
        xs = xT[:, c * NLOC:(c + 1) * NLOC]        # [D, NLOC]
        xh = np.stack([
            xs[:, h * HALF:(h + 1) * HALF].reshape(KT, P, HALF)
            for h in range(2)])                     # [2, KT, P, HALF]
        im = {"wt": WT,
              "xq0": np.ascontiguousarray(xh).astype(np.int8)}
        if not fast_gb:
            im["gam"] = np.ascontiguousarray(
                gammas.reshape(NUM_LAYERS - 1, KT, P).transpose(0, 2, 1))
            im["bet"] = np.ascontiguousarray(
                betas.reshape(NUM_LAYERS - 1, KT, P).transpose(0, 2, 1))
        in_maps.append(im)

    res = run_bass_kernel_spmd(nc, in_maps, list(range(NUM_CORES)),
                               trace=_trace)
    if _trace:
        kernel.last_exec_ns = res.exec_time_ns

    # gather: out[c] is [2, KT, P, HALF] -> per-core [D, NLOC] -> [N, D]
    full = np.empty((N_TOK, D), np.float32)
    for c in range(NUM_CORES):
        o = np.asarray(res.results[c]["out"], np.float32)  # [2, KT, P, HALF]
        for h in range(2):
            rows = o[h].reshape(D, HALF).T                 # [HALF, D]
            full[c * NLOC + h * HALF:c * NLOC + (h + 1) * HALF] = rows
    return full


kernel.last_exec_ns = None


# revision 6
# speedup vs baseline: 1.0024x; 1.0003x over previous
"""BitNet 4-layer MLP (8192x4096, ternary weights, int8-style activations)
on 8 Trainium2 NeuronCores — v2.

Strategy: pure data-parallel over the 8192-token dim (1024 tokens/core, no
collectives), activations transposed on chip ([feature, token]).  Weights are
ternary bf16 (exact), activations are int8-range integers in bf16 (exact),
PSUM accumulates fp32 (exact).  The 8192 N=512 bf16 matmuls per core are the
roofline (~213ns each); everything else is arranged to never stall the PE
FIFO:

  * LayerNorm stats are accumulated across output tiles on DVE/ACT
    (accS += h, accQ += Square(h)) so the PE only runs ONE f32r hi/lo
    ones-matmul pair per (layer, half) instead of 4 per output tile.
  * All auxiliary PE work (stats matmuls, mu/rstd broadcast matmuls) is
    emitted a few chunks INTO the next half's main-matmul stream, so its
    upstream DVE/ACT dependencies are always resolved before the PE FIFO
    reaches it.
  * The LN+quantize chain is fused to 4 DVE ops per tile (scale/offset rows
    folded with 1/in_scale, magic-number RNE quantization), spread 2 tiles
    per chunk across the next half's stream.
  * gamma==1/beta==0 (true for this model's inputs) removes the per-tile
    gamma/beta op; detected on host, with a generic fallback variant.
  * DRAM layouts are contiguous per DMA (half-major activations/outputs) to
    avoid the 1KB-row descriptor storm that made the baseline idle ~99us at
    startup.
"""

import numpy as np

NUM_CORES = 8
N_TOK, D = 8192, 4096
NUM_LAYERS = 4
P = 128                      # SBUF partitions
KT = D // P                  # 32 k-tiles per contraction
NLOC = N_TOK // NUM_CORES    # 1024 tokens per core
HALF = 512                   # token half-chunk (one PSUM bank @ fp32)
MAGIC = 12582912.0           # 1.5 * 2**23: fp32 add/sub does RNE-to-integer

_prog_cache = {}


def _install_drain_patch():
    """walrus CoreV3 rejects instructions carrying >~2 embedded sem waits
    ("Too many sync wait commands"). Tile's exit drain waits on the whole
    vector clock; spread its waits across trailing sync-engine nops."""
    import concourse.tile as tile
    import concourse.mybir as mybir
    from concourse.tile import ScopedClock

    if getattr(tile.TileContext, "_drain_patch_installed", False):
        return

    def _patched(self, tick_clock, wait_clock):
        nc = self.nc
        drain_inst = nc.sync.drain()
        wait_clock.add_sem_waits(
            drain_inst.ins, ScopedClock({None: tick_clock.global_clock})
        )
        si = drain_inst.ins.sync_info
        waits = list(si.on_wait or []) if si is not None else []
        if len(waits) > 1:
            si.on_wait = waits[:1]
            for w in waits[1:]:
                nop = nc.sync.nop(nofuse=True)
                nsi = nop.ins.sync_info
                if nsi is None:
                    nop.ins.sync_info = mybir.SyncInfo(on_wait=[w], on_update=[])
                else:
                    nsi.on_wait = [w]
        nc.all_engine_barrier()
        assert self.sems is not None
        popped = nc._tile_sem_poison_stack.pop()
        assert popped is self._sem_poison
        nc.clear_and_free_semaphores(list(self.sems.allocated().values()))
        nc.all_engine_barrier()

    tile.TileContext._drain_and_barrier = _patched
    tile.TileContext._drain_patch_installed = True


def _split_excess_waits(nc, maxw=1):
    """walrus's per-instruction sync-wait encodings hold few waits; hoist
    excess waits onto same-engine nops spliced immediately before the
    overloaded instruction (adjacent on the same queue, so ordering
    semantics are unchanged)."""
    import copy
    import concourse.mybir as mybir

    ctr = [0]
    proto = nc.sync.nop(nofuse=True)
    _NOP_PROTO = copy.deepcopy(proto.ins)
    _NOP_PROTO.sync_info = None

    def make_nop(proto_engine, waits):
        ctr[0] += 1
        nop = copy.deepcopy(_NOP_PROTO)
        nop.name = f"I-waitsplit-{ctr[0]}"
        nop.engine = proto_engine
        nop.sync_info = mybir.SyncInfo(on_wait=list(waits), on_update=[])
        return nop

    for bb in nc.m.functions[0].blocks:
        changed = False
        out = []
        for inst in bb.instructions:
            si = inst.sync_info
            waits = list(si.on_wait) if (si is not None and si.on_wait) else []
            if len(waits) > maxw and type(inst).__name__ != "InstISA":
                for i in range(0, len(waits) - maxw, maxw):
                    out.append(make_nop(inst.engine, waits[i:i + maxw]))
                si.on_wait = waits[len(waits) - maxw:]
                changed = True
            out.append(inst)
        if changed:
            bb.instructions = out
    return nc


def _build_program(s_deq, inv_in, fast_gb):
    """Build the per-core Bass program (identical across cores; data-parallel).

    s_deq[l]   = in_scale[l]*w_scale[l] as python floats (fp32-exact values)
    inv_in[l]  = 1/in_scale[l] likewise
    fast_gb    = True when gamma==1 and beta==0 (skip the per-tile op)
    """
    import concourse.bass as bass
    import concourse.mybir as mybir
    import concourse.tile as tile

    _install_drain_patch()
    dt = mybir.dt
    Alu = mybir.AluOpType
    Act = mybir.ActivationFunctionType

    nc = bass.Bass()
    W_d = nc.dram_tensor("wt", [NUM_LAYERS, KT, P, KT, P], dt.float8e4,
                         kind="ExternalInput")
    X_d = nc.dram_tensor("xq0", [2, KT, P, HALF], dt.int8,
                         kind="ExternalInput")
    O_d = nc.dram_tensor("out", [2, KT, P, HALF], dt.float32,
                         kind="ExternalOutput")
    if not fast_gb:
        G_d = nc.dram_tensor("gam", [NUM_LAYERS - 1, P, KT], dt.float32,
                             kind="ExternalInput")
        B_d = nc.dram_tensor("bet", [NUM_LAYERS - 1, P, KT], dt.float32,
                             kind="ExternalInput")

    f32, f32r, bf16 = dt.float32, dt.float32r, dt.bfloat16

    with tile.TileContext(nc) as tc:
        with (
            tc.tile_pool(name="xq", bufs=64) as xq_pool,
            tc.tile_pool(name="h", bufs=34) as h_pool,
            tc.tile_pool(name="w", bufs=4) as w_pool,
            tc.tile_pool(name="sq", bufs=3) as sq_pool,
            tc.tile_pool(name="acc", bufs=2) as acc_pool,
            tc.tile_pool(name="hr", bufs=4) as hr_pool,
            tc.tile_pool(name="bc", bufs=4) as bc_pool,
            tc.tile_pool(name="st", bufs=6) as st_pool,
            tc.tile_pool(name="gb", bufs=6) as gb_pool,
            tc.tile_pool(name="xi", bufs=8) as xi_pool,
            tc.tile_pool(name="const", bufs=1) as const_pool,
            tc.tile_pool(name="mmps", bufs=3, space="PSUM") as mm_ps,
            tc.tile_pool(name="stps", bufs=2, space="PSUM") as st_ps,
            tc.tile_pool(name="bcps", bufs=2, space="PSUM") as bc_ps,
        ):
            ones_f = const_pool.tile([P, 1], f32)
            nc.vector.memset(ones_f[:], 1.0)
            ones = const_pool.tile([P, 1], f32r)
            nc.vector.tensor_copy(ones[:], ones_f[:])
            eps = const_pool.tile([1, 1], f32)
            nc.vector.memset(eps[:], 1e-5)
            ones_row = const_pool.tile([1, P], f32)
            nc.vector.memset(ones_row[:], 1.0)
            ones_bf = const_pool.tile([P, 1], bf16)
            nc.vector.memset(ones_bf[:], 1.0)

            # state threaded between emission callbacks
            xq_tiles = {}     # (l, half, kt) -> bf16 [P, HALF]
            h_tiles = {}      # (l, half, ot) -> f32 [P, HALF]
            accs = {}         # (l, half) -> (accS, accQ)
            stps = {}         # (l, half) -> (S_ps, Q_ps)
            rows = {}         # (l, half) -> (a_row, b_row)
            bcs = {}          # (l, half) -> (aB, bB)
            gbt = {}          # l -> (gam [P,KT], bet [P,KT])

            # PE warmup: HAM un-throttles after ~3.4us of sustained matmul
            # activity; burn tiny matmuls on the const tile while the first
            # input DMAs are in flight so the real stream starts at 2.4GHz.
            warm_ps = st_ps.tile([1, 1], f32, tag="stps")
            for _ in range(200):
                nc.tensor.matmul(warm_ps[:], ones_bf[:], ones_bf[:],
                                 start=True, stop=True,
                                 skip_group_check=True)

            # first two weight tiles ahead of everything else: their
            # descriptors land at the head of every DMA queue, so the PE can
            # start chunk 0 as soon as the first xq tiles trickle in.
            def load_w(l, ot):
                w = w_pool.tile([P, KT, P], dt.float8e4, tag="w")
                nc.sync.dma_start(w[:], W_d[l, ot])
                return w

            pre_w = {}

            # initial activation DMAs: half 0 up-front (first chunks need all
            # 32 of them); half 1 interleaved into the first 32 chunks.
            def load_xq0(half, kt):
                ti = xi_pool.tile([P, HALF], dt.int8, tag="xi")
                nc.sync.dma_start(ti[:], X_d[half, kt])
                t = xq_pool.tile([P, HALF], bf16, tag="xq")
                nc.vector.tensor_copy(t[:], ti[:])
                xq_tiles[(0, half, kt)] = t

            pre_w[0] = load_w(0, 0)
            pre_w[1] = load_w(0, 1)
            for kt in range(KT):
                load_xq0(0, kt)

            if not fast_gb:
                for l in range(NUM_LAYERS - 1):
                    g = gb_pool.tile([P, KT], f32, tag="gb")
                    nc.sync.dma_start(g[:], G_d[l])
                    b = gb_pool.tile([P, KT], f32, tag="gb")
                    nc.sync.dma_start(b[:], B_d[l])
                    gbt[l] = (g, b)

            def emit_chunk(l, half, ot):
                if l == 0 and half == 0:
                    # stream in half 1 of the initial activations
                    load_xq0(1, ot)
                if l == 0 and half == 0 and ot in pre_w:
                    w = pre_w.pop(ot)
                else:
                    w = load_w(l, ot)
                ps = mm_ps.tile([P, HALF], f32, tag="mmps")
                for kt in range(KT):
                    nc.tensor.matmul(
                        ps[:], w[:, kt, :], xq_tiles[(l, half, kt)][:],
                        start=(kt == 0), stop=(kt == KT - 1),
                        skip_group_check=True)
                if l < NUM_LAYERS - 1:
                    h_t = h_pool.tile([P, HALF], f32, tag="h")
                    nc.scalar.activation(h_t[:], ps[:], Act.Relu,
                                         scale=float(s_deq[l]))
                    sq = sq_pool.tile([P, HALF], f32, tag="sq")
                    nc.scalar.activation(sq[:], h_t[:], Act.Square)
                    if ot == 0:
                        accS = acc_pool.tile([P, HALF], f32, tag="accS")
                        nc.vector.tensor_copy(accS[:], h_t[:])
                        accQ = acc_pool.tile([P, HALF], f32, tag="accQ")
                        nc.vector.tensor_copy(accQ[:], sq[:])
                        accs[(l, half)] = (accS, accQ)
                    else:
                        accS, accQ = accs[(l, half)]
                        nc.vector.tensor_tensor(accS[:], accS[:], h_t[:],
                                                op=Alu.add)
                        nc.vector.tensor_tensor(accQ[:], accQ[:], sq[:],
                                                op=Alu.add)
                    h_tiles[(l, half, ot)] = h_t
                else:
                    h_t = h_pool.tile([P, HALF], f32, tag="h")
                    nc.scalar.activation(h_t[:], ps[:], Act.Copy,
                                         scale=float(s_deq[l]))
                    nc.sync.dma_start(O_d[half, ot], h_t[:])

            def emit_stats(l, half):
                # partition-dim sums of accS/accQ via f32r hi/lo ones-matmuls
                # (full fp32 accuracy, 4 PE instructions total)
                accS, accQ = accs.pop((l, half))
                S_ps = st_ps.tile([1, HALF], f32, tag="stps")
                Q_ps = st_ps.tile([1, HALF], f32, tag="stps")
                for acc, ps_ in ((accS, S_ps), (accQ, Q_ps)):
                    hi = hr_pool.tile([P, HALF], f32r, tag="hr")
                    nc.vector.tensor_copy(hi[:], acc[:])
                    lo = hr_pool.tile([P, HALF], f32r, tag="hr")
                    nc.vector.tensor_tensor(lo[:], acc[:],
                                            hi[:].bitcast(f32),
                                            op=Alu.subtract)
                    nc.tensor.matmul(ps_[:], ones[:], hi[:],
                                     start=True, stop=False,
                                     skip_group_check=True)
                    nc.tensor.matmul(ps_[:], ones[:], lo[:],
                                     start=False, stop=True,
                                     skip_group_check=True)
                stps[(l, half)] = (S_ps, Q_ps)

            def emit_rows(l, half):
                S_ps, Q_ps = stps.pop((l, half))
                mu = st_pool.tile([1, HALF], f32, tag="st")
                nc.vector.tensor_scalar_mul(mu[:], S_ps[:], 1.0 / D)
                q = st_pool.tile([1, HALF], f32, tag="st")
                nc.vector.tensor_scalar_mul(q[:], Q_ps[:], 1.0 / D)
                var = st_pool.tile([1, HALF], f32, tag="st")
                nc.vector.tensor_tensor(var[:], mu[:], mu[:], op=Alu.mult)
                nc.vector.tensor_tensor(var[:], q[:], var[:], op=Alu.subtract)
                std = st_pool.tile([1, HALF], f32, tag="st")
                nc.scalar.activation(std[:], var[:], Act.Sqrt, bias=eps[:])
                rstd = st_pool.tile([1, HALF], f32, tag="st")
                nc.vector.reciprocal(rstd[:], std[:])
                a = st_pool.tile([1, HALF], f32, tag="st")
                if fast_gb:
                    # fold 1/in_scale of the next layer into the LN scale
                    nc.vector.tensor_scalar_mul(a[:], rstd[:],
                                                float(inv_in[l + 1]))
                else:
                    nc.vector.tensor_copy(a[:], rstd[:])
                b = st_pool.tile([1, HALF], f32, tag="st")
                nc.vector.tensor_tensor(b[:], mu[:], a[:], op=Alu.mult)
                rows[(l, half)] = (a, b)

            def emit_bcast(l, half):
                a, b = rows.pop((l, half))
                out = []
                for r in (a, b):
                    r_ps = bc_ps.tile([P, HALF], f32, tag="bcps")
                    nc.tensor.matmul(r_ps[:], ones_row[:], r[:],
                                     start=True, stop=True,
                                     skip_group_check=True)
                    rB = bc_pool.tile([P, HALF], f32, tag="bc")
                    nc.scalar.activation(rB[:], r_ps[:], Act.Copy)
                    out.append(rB)
                bcs[(l, half)] = tuple(out)

            def emit_ln(l, half, fts):
                aB, bB = bcs[(l, half)]
                inv = float(inv_in[l + 1])
                for ft in fts:
                    h_t = h_tiles.pop((l, half, ft))
                    nc.vector.tensor_tensor(h_t[:], h_t[:], aB[:],
                                            op=Alu.mult)
                    nc.vector.tensor_tensor(h_t[:], h_t[:], bB[:],
                                            op=Alu.subtract)
                    if not fast_gb:
                        g, bta = gbt[l]
                        nc.vector.tensor_scalar(h_t[:], h_t[:],
                                                g[:, ft:ft + 1],
                                                bta[:, ft:ft + 1],
                                                op0=Alu.mult, op1=Alu.add)
                        nc.vector.tensor_scalar(h_t[:], h_t[:], inv, MAGIC,
                                                op0=Alu.mult, op1=Alu.add)
                        nc.vector.tensor_scalar(h_t[:], h_t[:], MAGIC + 127.0,
                                                MAGIC - 128.0, op0=Alu.min,
                                                op1=Alu.max)
                        xq_t = xq_pool.tile([P, HALF], bf16, tag="xq")
                        nc.vector.tensor_scalar_add(xq_t[:], h_t[:], -MAGIC)
                    else:
                        nc.vector.tensor_scalar(h_t[:], h_t[:], MAGIC,
                                                MAGIC + 127.0,
                                                op0=Alu.add, op1=Alu.min)
                        xq_t = xq_pool.tile([P, HALF], bf16, tag="xq")
                        nc.vector.tensor_scalar(xq_t[:], h_t[:],
                                                MAGIC - 128.0, MAGIC,
                                                op0=Alu.max, op1=Alu.subtract)
                    xq_tiles[(l + 1, half, ft)] = xq_t

            # emission schedule: 256 main chunks with aux work interleaved a
            # few chunks into the following stream so the PE FIFO never waits
            chunks = [(l, half, ot)
                      for l in range(NUM_LAYERS)
                      for half in range(2)
                      for ot in range(KT)]
            aux = {}
            for l in range(NUM_LAYERS - 1):
                for half in range(2):
                    g_end = (l * 2 + half) * KT + KT - 1
                    aux.setdefault(g_end + 2, []).append(
                        lambda l=l, h=half: emit_stats(l, h))
                    aux.setdefault(g_end + 3, []).append(
                        lambda l=l, h=half: emit_rows(l, h))
                    aux.setdefault(g_end + 4, []).append(
                        lambda l=l, h=half: emit_bcast(l, h))
                    for i in range(16):
                        aux.setdefault(g_end + 5 + i, []).append(
                            lambda l=l, h=half, i=i:
                                emit_ln(l, h, [2 * i, 2 * i + 1]))

            # aux goes BEFORE its chunk: its DVE ops have ready inputs, while
            # the chunk's acc-adds are gated on the chunk's matmul group —
            # emitting them last keeps the (strict-FIFO) DVE from idling on
            # them with runnable LN work queued behind.
            for g, (l, half, ot) in enumerate(chunks):
                for fn in aux.pop(g, []):
                    fn()
                emit_chunk(l, half, ot)
            assert not aux, f"unemitted aux work: {sorted(aux)}"

    _split_excess_waits(nc)
    return nc


def kernel(x, Ws, w_scales, in_scales, gammas, betas, _trace=False):
    import ml_dtypes
    from concourse.bass_utils import run_bass_kernel_spmd

    f32 = np.float32
    C = f32(MAGIC)
    x = np.asarray(x, f32)
    Ws = np.asarray(Ws, f32)
    w_scales = np.asarray(w_scales, f32)
    in_scales = np.asarray(in_scales, f32)
    gammas = np.asarray(gammas, f32)
    betas = np.asarray(betas, f32)

    # ---- host prep (offline-weight-style preprocessing) ----
    # ternary quantize weights; XLA divides by reciprocal-multiply and
    # rounds nearest-even, both reproduced here bitwise.
    WT = np.empty((NUM_LAYERS, KT, P, KT, P), ml_dtypes.float8_e4m3fn)
    for l in range(NUM_LAYERS):
        wq = ((Ws[l] * (f32(1.0) / w_scales[l])) + C) - C
        wq = np.clip(wq, -1.0, 1.0).astype(f32)
        # WT[l, ot, kp, kt, o] = wq[ot*128+o, kt*128+kp]
        t = wq.reshape(KT, P, KT, P)          # [ot, o, kt, kp]
        WT[l] = t.transpose(0, 3, 2, 1).astype(ml_dtypes.float8_e4m3fn)

    xq0 = ((x * (f32(1.0) / in_scales[0])) + C) - C
    xq0 = np.clip(xq0, -128.0, 127.0).astype(f32)
    xT = np.ascontiguousarray(xq0.T)           # [D, N]

    fast_gb = bool(np.all(gammas == 1.0) and np.all(betas == 0.0))
    s_deq = [float(in_scales[l] * w_scales[l]) for l in range(NUM_LAYERS)]
    inv_in = [float(f32(1.0) / in_scales[l]) for l in range(NUM_LAYERS)]

    key = (tuple(s_deq), tuple(inv_in), fast_gb)
    if key not in _prog_cache:
        _prog_cache[key] = _build_program(s_deq, inv_in, fast_gb)
    nc = _prog_cache[key]

    in_maps = []
    for c in range(NUM_CORES):
        xs = xT[:, c * NLOC:(c + 1) * NLOC]        # [D, NLOC]
        xh = np.stack([
            xs[:, h * HALF:(h + 1) * HALF].reshape(KT, P, HALF)
            for h in range(2)])                     # [2, KT, P, HALF]
        im = {"wt": WT,
              "xq0": np.ascontiguousarray(xh).astype(np.int8)}
        if not fast_gb:
            im["gam"] = np.ascontiguousarray(
                gammas.reshape(NUM_LAYERS - 1, KT, P).transpose(0, 2, 1))
            im["bet"] = np.ascontiguousarray(
                betas.reshape(NUM_LAYERS - 1, KT, P).transpose(0, 2, 1))
        in_maps.append(im)

    res = run_bass_kernel_spmd(nc, in_maps, list(range(NUM_CORES)),
                               trace=_trace)
    if _trace:
        kernel.last_exec_ns = res.exec_time_ns

    # gather: out[c] is [2, KT, P, HALF] -> per-core [D, NLOC] -> [N, D]
    full = np.empty((N_TOK, D), np.float32)
    for c in range(NUM_CORES):
        o = np.asarray(res.results[c]["out"], np.float32)  # [2, KT, P, HALF]
        for h in range(2):
            rows = o[h].reshape(D, HALF).T                 # [HALF, D]
            full[c * NLOC + h * HALF:c * NLOC + (h + 1) * HALF] = rows
    return full


kernel.last_exec_ns = None
